# revision 1
# baseline (speedup 1.0000x reference)
"""Trainium2 Bass kernel for MinibatchDiscrimination.

Reference computation:
    M = (x @ T.reshape(A, B*C)).reshape(N, B, C)       x:[2048,512] T:[512,64,16]
    O[i, b] = sum_j exp(-sum_c |M[j,b,c] - M[i,b,c]|)   O:[2048,64]

Strategy (8 cores, shard output rows i):
  - Host feeds each core XTm = [roll(x.T, -core*256, axis=1) | T.reshape(512,1024)]
    as ONE tensor: a single input DMA (one HWDGE queue sem) that matmuls may
    wait on directly (matmul weight-loads can carry only ONE sem wait).
    The rotation makes the core's 256 shard rows columns 0..255 of its own
    M^T — the program is identical across cores (pure SPMD), and the
    i-column used for the subtrahend is bit-identical to the j-column it
    meets on the diagonal (exp(0) = 1 exactly).
  - PE builds MT = (x@T)^T as [bc=1024, j=2048] in fp32, rounded to bf16.
  - Main loop over 128 i-pairs: ACT (Abs with per-partition bias) and DVE
    (tensor_scalar add-neg + abs_max 0) split the |M_j - M_i| work; PE
    contracts c (16) with a 0/1 block-indicator matmul into PSUM
    [2i x 64b, j]; ACT does fused Exp(-D) (in-place on PSUM) + free-dim
    accumulate -> one column of O per pair.

Sync-wait discipline (walrus limits: matmul 1 wait, others 2):
  - every matmul's deps must collapse onto a single engine sem. ACT produces
    the first-consumed A tiles in steady state so the psum-WAR wait (ACT exp
    of 2 pairs ago) is subsumed by the newer ACT A-ready wait; the very first
    psum tile instead consumes DVE-produced tiles first (its only other deps
    are the DVE Sbig copy / MT copies).
  - input/LHS tensors stay resident (no SBUF address reuse -> no WAR fan-in).
"""

import numpy as np
import ml_dtypes

N, A_DIM, B, C = 2048, 512, 64, 16
BC = B * C
N_CORES = 8
SHARD = N // N_CORES      # 256
PAIRS = SHARD // 2        # 128
N_ACT = 6                 # of the 16 absdiff instrs per (pair, jhalf), how many go to ACT

_CACHE = {}


def _build_nc(npairs=PAIRS, n_act=N_ACT, debug_mt=False):
    from contextlib import ExitStack
    import concourse.bass as bass
    import concourse.mybir as mybir
    from concourse.tile import TileContext, add_dep_helper

    f32 = mybir.dt.float32
    bf16 = mybir.dt.bfloat16
    Abs = mybir.ActivationFunctionType.Abs
    Exp = mybir.ActivationFunctionType.Exp
    add_op = mybir.AluOpType.add
    band_op = mybir.AluOpType.bitwise_and

    nc = bass.Bass("TRN2", target_bir_lowering=False, debug=False)
    # columns: [xT_rot (2048) | Tm (1024) | Sb-as-f32 (60, rows 0..127) | absmask (1)]
    XTm = nc.dram_tensor("XTm", [A_DIM, N + BC + 61], f32, kind="ExternalInput").ap()
    O = nc.dram_tensor("O", [SHARD, B], f32, kind="ExternalOutput").ap()
    MT_dbg = None
    if debug_mt:
        MT_dbg = nc.dram_tensor("MT_dbg", [128, 8, N], bf16, kind="ExternalOutput").ap()

    with TileContext(nc) as tc, ExitStack() as ctx:
        singles = ctx.enter_context(tc.tile_pool(name="singles", bufs=1))
        Sbig = singles.tile([128, 120], bf16, name="Sbig", tag="Sbig")

        XAll = singles.tile([128, 4, N + BC + 61], f32, name="XAll", tag="XAll")  # 48KB/part
        nc.sync.dma_start(out=XAll, in_=XTm.rearrange("(k p) n -> p k n", p=128))
        nc.scalar.copy(Sbig, XAll[:, 0, N + BC:N + BC + 60].bitcast(bf16))

        MT = singles.tile([128, 8, N], bf16, name="MT", tag="MT")                  # 32KB/part
        negMTi = singles.tile([128, 8, SHARD], f32, name="negMTi", tag="negMTi")   # 8KB/part
        O_sb = singles.tile([128, PAIRS], f32, name="O_sb", tag="O_sb")
        junk1 = singles.tile([1, PAIRS], f32, name="junk1", tag="junk1")
        junkA = singles.tile([1, PAIRS], f32, name="junkA", tag="junkA")
        junkA2 = singles.tile([1, PAIRS], f32, name="junkA2", tag="junkA2")
        junkE = singles.tile([1, 2 * PAIRS], f32, name="junkE", tag="junkE")
        junkD = singles.tile([1, 2 * PAIRS], f32, name="junkD", tag="junkD")
        junkD2 = singles.tile([1, PAIRS], f32, name="junkD2", tag="junkD2")
        u16 = mybir.dt.uint16
        mask = singles.tile([128, 1], u16, name="mask", tag="mask")
        nc.vector.memset(mask, 0x7FFF)   # bf16 sign-bit clear mask

        # ---- prologue: MT = (x @ T)^T = Tm^T @ xT, fp32, rounded to bf16
        # all copies on ACT: constants then have a single producer engine, so
        # downstream single-wait instructions (matmul, tensor_scalar) only
        # ever need one ACT sem to cover them.
        ppsum = ctx.enter_context(tc.tile_pool(name="ppsum", bufs=4, space="PSUM"))
        for m in range(8):
            for n in range(4):
                ps = ppsum.tile([128, 512], f32, name="mmps", tag="mmps")
                for k in range(4):
                    nc.tensor.matmul(
                        ps,
                        XAll[:, k, N + 128 * m:N + 128 * (m + 1)],
                        XAll[:, k, 512 * n:512 * (n + 1)],
                        start=(k == 0), stop=(k == 3),
                    )
                nc.scalar.copy(MT[:, m, 512 * n:512 * (n + 1)], ps)
        for m in range(8):
            # negate the *rounded* values so ACT's bias matches DVE's subtrahend
            nc.scalar.mul(negMTi[:, m, :], MT[:, m, 0:SHARD], -1.0)

        if debug_mt:
            nc.sync.dma_start(out=MT_dbg, in_=MT)

        # ---- main loop
        with tc.tile_pool(name="apool", bufs=2) as apool, \
             tc.tile_pool(name="mpsum", bufs=2, space="PSUM") as mpsum, \
             tc.tile_pool(name="accp", bufs=4) as accp:
            prev_mm_last = None      # end of pair q-1
            prev2_mm_last = None     # end of pair q-2
            prev_jh0_mm = None       # last matmul of (q-1, jh0)
            prev_add = None
            groups = []              # per psum group: probe_b / exp handles
            for q in range(npairs):
                # --- per-pair wait-budget absorbers (see module docstring):
                # ACT absdiff tiles are [128,2048] bufs=2 -> WAR vs pair q-2 readers
                absorber_a = nc.scalar.copy(junkA[:, q:q + 1], negMTi[0:1, 0, q:q + 1])
                if prev2_mm_last is not None:
                    add_dep_helper(absorber_a.ins, prev2_mm_last.ins, sync=True,
                                   reason="advance ACT PE-clock for absdiff wait budget")
                # DVE absdiff tiles are [128,1024] bufs=2 -> WAR vs (q-1, same jh)
                absorber_d0 = nc.vector.tensor_copy(junk1[:, q:q + 1], mask[0:1, 0:1])
                if prev_jh0_mm is not None:
                    add_dep_helper(absorber_d0.ins, prev_jh0_mm.ins, sync=True,
                                   reason="advance DVE PE-clock (jh0) for TS wait budget")
                absorber_d1 = nc.vector.tensor_copy(junkD2[:, q:q + 1], mask[0:1, 0:1])
                if prev_mm_last is not None:
                    add_dep_helper(absorber_d1.ins, prev_mm_last.ins, sync=True,
                                   reason="advance DVE PE-clock (jh1) for TS wait budget")

                # --- absdiff producers, one [128,2048] ACT op or 2x [128,1024]
                # DVE op-pairs per (t, ii)
                a_tiles = {}
                first_act_inst = None
                for t in range(8):
                    for ii in range(2):
                        col = 2 * q + ii
                        idx = t * 2 + ii
                        if idx < n_act:
                            At = apool.tile([128, 2048], bf16, name=f"Aa{t}_{ii}",
                                            tag=f"Aa{t}_{ii}", bufs=2)
                            act_inst = nc.scalar.activation(
                                At, MT[:, t, :], Abs, bias=negMTi[:, t, col:col + 1])
                            add_dep_helper(act_inst.ins, absorber_a.ins, sync=False,
                                           reason="order absdiff after PE-clock absorber")
                            if first_act_inst is None:
                                first_act_inst = act_inst
                            a_tiles[(t, ii)] = (At, 0)
                        else:
                            Ad0 = apool.tile([128, 1024], bf16, name=f"Ad{t}_{ii}_0",
                                             tag=f"Ad{t}_{ii}_0", bufs=1)
                            Ad1 = apool.tile([128, 1024], bf16, name=f"Ad{t}_{ii}_1",
                                             tag=f"Ad{t}_{ii}_1", bufs=1)
                            for jh, Ad, ab in ((0, Ad0, absorber_d0), (1, Ad1, absorber_d1)):
                                ts_inst = nc.vector.tensor_scalar(
                                    Ad, MT[:, t, jh * 1024:(jh + 1) * 1024],
                                    negMTi[:, t, col:col + 1], None, op0=add_op)
                                add_dep_helper(ts_inst.ins, ab.ins, sync=False,
                                               reason="order TS after PE-clock absorber")
                                # abs: clear the bf16 sign bit
                                nc.vector.tensor_scalar(
                                    Ad.bitcast(u16), Ad.bitcast(u16), mask, None,
                                    op0=band_op)
                            a_tiles[(t, ii)] = ((Ad0, Ad1), 1)

                # --- two psum groups (jh halves), c-contraction + exp
                parts = []
                for jh in range(2):
                    ps = mpsum.tile([128, 1024], f32, name="ps", tag="ps")
                    probe_a = None
                    probe_b = None
                    jh_last_mm = None
                    for t in range(8):
                        lhsT = Sbig[:, 56 - 8 * t:120 - 8 * t]
                        for ii in range(2):
                            idx = t * 2 + ii
                            tile, kind = a_tiles[(t, ii)]
                            for nn in range(2):
                                if kind == 0:
                                    rhs = tile[:, jh * 1024 + 512 * nn:jh * 1024 + 512 * (nn + 1)]
                                else:
                                    rhs = tile[jh][:, 512 * nn:512 * (nn + 1)]
                                mm_last = nc.tensor.matmul(
                                    ps[64 * ii:64 * (ii + 1), 512 * nn:512 * (nn + 1)],
                                    lhsT, rhs, start=(t == 0), stop=(t == 7),
                                )
                                if idx == n_act - 1:
                                    if nn == 0:
                                        probe_a = mm_last
                                    else:
                                        probe_b = mm_last
                    jh_last_mm = mm_last
                    # pre-advance PE's ACT clock (matmul 1-wait limit):
                    #   probe_a(G) -> exp(G-1)        (psum bank WAW writer)
                    #   probe_b(G-1) -> first_act(q)  (next pair's A-ready)
                    if groups:
                        prev = groups[-1]
                        if jh == 0 and prev['probe_b'] is not None and first_act_inst is not None:
                            add_dep_helper(prev['probe_b'].ins, first_act_inst.ins,
                                           sync=True,
                                           reason="pre-advance PE ACT-clock: next pair A-ready")
                        if probe_a is not None and prev['exp'] is not None:
                            add_dep_helper(probe_a.ins, prev['exp'].ins, sync=True,
                                           reason="pre-advance PE ACT-clock: prev group exp")
                    pt = accp.tile([128, 1], f32, name=f"pt{jh}", tag=f"pt{jh}")
                    # pre-absorb the pt-slot WAR (DVE tensor_add of an earlier
                    # pair) and the psum-RAW PE wait, so exp only carries its
                    # own-engine WAW wait (1-wait limit)
                    g = 2 * q + jh
                    if prev_add is not None:
                        absorber_pd = nc.scalar.copy(junkD[:, g:g + 1], negMTi[0:1, 2, g:g + 1])
                        add_dep_helper(absorber_pd.ins, prev_add.ins, sync=True,
                                       reason="advance ACT DVE-clock for exp pt WAR")
                    absorber_e = nc.scalar.copy(junkE[:, g:g + 1], negMTi[0:1, 1, g:g + 1])
                    add_dep_helper(absorber_e.ins, mm_last.ins, sync=True,
                                   reason="advance ACT PE-clock for exp wait budget")
                    # exp in place on the psum tile (out only needed for accum)
                    exp_inst = nc.scalar.activation(ps, ps, Exp, scale=-1.0, accum_out=pt)
                    add_dep_helper(exp_inst.ins, absorber_e.ins, sync=False,
                                   reason="order exp after PE-clock absorber")
                    if prev_add is not None:
                        add_dep_helper(exp_inst.ins, absorber_pd.ins, sync=False,
                                       reason="order exp after DVE-clock absorber")
                    groups.append({'probe_b': probe_b, 'exp': exp_inst})
                    parts.append(pt)
                    if jh == 0:
                        pair_jh0_mm = jh_last_mm
                prev_add = nc.vector.tensor_add(O_sb[:, q:q + 1], parts[0], parts[1])
                prev2_mm_last = prev_mm_last
                prev_mm_last = mm_last
                prev_jh0_mm = pair_jh0_mm
            dma_o = nc.sync.dma_start(
                out=O.rearrange("(q i2) b -> i2 b q", i2=2),
                in_=O_sb,
            )
    # The kernel-tail gather Drain aggregates one wait per active proc (3
    # engines + 2 DMA queues = 5), exceeding the CTRL struct's wait slots.
    # Every proc's completion is transitively dominated by the output DMA
    # (out-DMA <- DVE adds <- ACT exp <- PE matmuls <- input DMA), so waiting
    # only on the output queue sem is sufficient.
    out_upd = {(u.ant_name) for u in dma_o.ins.sync_info.on_update}
    for f in nc.m.functions:
        for bb in f.blocks:
            for ins in bb.instructions:
                si = getattr(ins, 'sync_info', None)
                if si is None or ins.opcode != 'Drain':
                    continue
                if len(si.on_wait) > 2:
                    kept = [w for w in si.on_wait if w.ant_name in out_upd]
                    assert kept, f"drain {ins.name} has no output-queue wait"
                    si.on_wait = kept
    return nc


def _host_inputs(x, T):
    xT = np.ascontiguousarray(np.asarray(x, dtype=np.float32).T)        # [512, 2048]
    Tm = np.ascontiguousarray(np.asarray(T, dtype=np.float32).reshape(A_DIM, BC))
    Sb = np.zeros((128, 120), dtype=ml_dtypes.bfloat16)
    for p in range(128):
        Sb[p, 56 + p // 16] = 1
    Sb_f32 = np.zeros((A_DIM, 60), dtype=np.float32)
    Sb_f32[:128] = Sb.view(np.uint16).reshape(128, 60, 2).view(np.uint32).reshape(128, 60).view(np.float32)
    mask = np.full((A_DIM, 1), 0x7FFFFFFF, dtype=np.uint32).view(np.float32)
    in_maps = []
    for c in range(N_CORES):
        xT_rot = np.roll(xT, -c * SHARD, axis=1)
        XTmc = np.ascontiguousarray(np.concatenate([xT_rot, Tm, Sb_f32, mask], axis=1))
        in_maps.append({"XTm": XTmc})
    return in_maps


def run(x, T, npairs=PAIRS, n_act=N_ACT, trace=False, debug_mt=False):
    from concourse.bass_utils import run_bass_kernel_spmd

    key = (npairs, n_act, debug_mt)
    nc = _CACHE.get(key)
    if nc is None:
        nc = _build_nc(npairs, n_act, debug_mt)
        _CACHE[key] = nc
    in_maps = _host_inputs(x, T)
    res = run_bass_kernel_spmd(nc, in_maps, list(range(N_CORES)), trace=trace)
    O = np.concatenate([res.results[c]["O"] for c in range(N_CORES)], axis=0)
    return np.asarray(O, dtype=np.float32), res


def kernel(x, T):
    O, _ = run(x, T)
    return O

